# revision 1
# baseline (speedup 1.0000x reference)
"""Causal self-attention (B=4, T=2048, C=1024, H=16) on 8 Trainium2 NeuronCores.

Core index = 2*batch + head_group: each core owns one batch element and 8 of
the 16 heads (tensor-parallel split of c_attn output dim / c_proj input dim).
Each core emits a partial projection out^T [C, T]; the host sums the two
head-group partials per batch and adds the bias terms.

fp16 datapath (fp32 PSUM accumulation everywhere, fp32 softmax denominator):
  x, W_qk, W_v, W_p are cast to fp16 on the host. fp16 weights get FWL
  (fast weight load), making per-matmul LDWEIGHTS ~4x cheaper than fp32/f32r,
  and x^T comes from a single XBAR DMA-transpose instead of 128 PE transposes.

Per-core pipeline (Tile-scheduled, phases overlap via data deps):
  A: xT = DMA-transpose(x)                       [fp16]
  B: qkT[co, tn] = W_qk^T x^T; v = x @ W_v       [fp16 matmuls, fp32 psum]
  C per head h, per 512-wide i-chunk ic:
     S^T[j, i] = k_h^T q_h   (psum groups of 2 j-tiles [128, 2, 512])
     P = exp(S^T / 8)        (one ACT op per group -> fp16)
     causal mask on diagonal groups (DVE, precomputed mask tiles)
     U'^T [65, i] (+)= [v|1]^T P^T  over j-tiles (ones column => rowsum row 64)
     yT[hd, i] = U'^T[0:64] * bcast(1/rowsum)  (ACT copies, gpsimd
                 partition_broadcast, DVE reciprocal + multiply) -> fp16
  D: out^T = W_p^T yT -> fp32 psum -> ACT copy -> DMA
"""

import numpy as np

import concourse.bass as bass
import concourse.mybir as mybir
import concourse.tile as tile
from concourse import bacc, bass_utils

B, T, C, H = 4, 2048, 1024, 16
HD = C // H          # 64 head dim
N_CORES = 8
HG = H // 2          # 8 heads per core
CL = HG * HD         # 512 local width of q/k/v
TT = T // 128        # 16 t-tiles
CB = C // 128        # 8 c-tiles
DB = CL // 128       # 4 local-hd tiles
NIC = T // 512       # i-chunks (4)

f32 = mybir.dt.float32
f16 = mybir.dt.float16

_PROG_CACHE = {}


def _emit(tc, aps):
    nc = tc.nc
    Exp = mybir.ActivationFunctionType.Exp

    x_ap = aps["x"]
    wqk_ap = aps["wqk"]
    wv_ap = aps["wv"]
    wp_ap = aps["wp"]
    bqk_ap = aps["bqk"]
    masks_ap = aps["masks"]
    outT_ap = aps["outT"]

    from contextlib import ExitStack

    with ExitStack() as outer:
        const = outer.enter_context(tc.tile_pool(name="const", bufs=1))
        p_xT = outer.enter_context(tc.tile_pool(name="xT", bufs=1))
        p_qkT = outer.enter_context(tc.tile_pool(name="qkT", bufs=1))
        p_v = outer.enter_context(tc.tile_pool(name="vv", bufs=1))
        p_yT = outer.enter_context(tc.tile_pool(name="yT", bufs=1))
        p_w = outer.enter_context(tc.tile_pool(name="wsb", bufs=1))

        # critical-path DMAs on sync/HWDGE: wqk then x chunks
        wqk_sb = p_w.tile([128, CB, CB * 128], f16)  # [c-part, cb, co*128+q]
        nc.sync.dma_start(wqk_sb[:], wqk_ap.rearrange("(cb p) n -> p cb n", p=128))
        xT = p_xT.tile([128, CB, T], f16)
        for tn in range(NIC):
            nc.sync.dma_start_transpose(
                xT[:, :, tn * 512 : (tn + 1) * 512],
                x_ap[tn * 512 : (tn + 1) * 512, :],
            )
        wv_sb = p_w.tile([128, CB, CL], f16)
        nc.sync.dma_start(wv_sb[:], wv_ap.rearrange("(cb p) n -> p cb n", p=128))
        wp_sb = p_w.tile([128, DB, C], f16)
        nc.sync.dma_start(wp_sb[:], wp_ap.rearrange("(db p) c -> p db c", p=128))
        masks = const.tile([128, 4, 512], f16)   # 1 where j > i (to be masked)
        nc.gpsimd.dma_start(masks[:], masks_ap)
        negI = const.tile([128, 128], f16)
        nc.gpsimd.dma_start(negI[:], aps["negI"])
        bqk = const.tile([128, CB], f32)
        nc.gpsimd.dma_start(bqk[:], bqk_ap.rearrange("co p -> p co"))

        # per-(co, tn) qkT tiles, per-jt v' tiles, per-tn yT tiles
        qkT = {}
        for co in range(CB):
            for tn in range(NIC):
                qkT[(co, tn)] = p_qkT.tile(
                    [128, 512], f16, tag=f"qkT_{co}_{tn}", name=f"qkT_{co}_{tn}"
                )
        vv = {}
        for jt in range(TT):
            vv[jt] = p_v.tile([128, HG, HD + 1], f16, tag=f"vv_{jt}", name=f"vv_{jt}")
            nc.vector.memset(vv[jt][:, :, HD : HD + 1], 1.0)
        yTn = {}
        for tn in range(NIC):
            yTn[tn] = p_yT.tile([128, DB, 512], f16, tag=f"yT_{tn}", name=f"yT_{tn}")

        with ExitStack() as s_all:
            ps_ab = ExitStack()
            ps_mm = ps_ab.enter_context(tc.tile_pool(name="ps_mm", bufs=4, space="PSUM"))

            # ---- B: qkv projections, tn-major so attention can start early ---
            for tn in range(NIC):
                for co in range(CB):
                    ps = ps_mm.tile([128, 512], f32, tag="mm")
                    for cb in range(CB):
                        nc.tensor.matmul(
                            ps[:],
                            wqk_sb[:, cb, co * 128 : (co + 1) * 128],
                            xT[:, cb, tn * 512 : (tn + 1) * 512],
                            start=(cb == 0),
                            stop=(cb == CB - 1),
                        )
                    nc.vector.tensor_scalar_add(qkT[(co, tn)][:], ps[:], bqk[:, co : co + 1])
                for u in range(4):
                    tt = tn * 4 + u
                    ps = ps_mm.tile([128, CL], f32, tag="mm")
                    for cb in range(CB):
                        nc.tensor.matmul(
                            ps[:],
                            xT[:, cb, tt * 128 : (tt + 1) * 128],
                            wv_sb[:, cb, :],
                            start=(cb == 0),
                            stop=(cb == CB - 1),
                        )
                    nc.scalar.activation(
                        vv[tt][:, :, 0:HD],
                        ps.rearrange("p (h d) -> p h d", d=HD),
                        mybir.ActivationFunctionType.Copy,
                    )

            ps_ab.close()  # free A/B psum banks

            # ---- C: attention + interleaved projection -----------------------
            p_p = s_all.enter_context(tc.tile_pool(name="pp", bufs=12))
            p_usb = s_all.enter_context(tc.tile_pool(name="usb", bufs=3))
            p_rb = s_all.enter_context(tc.tile_pool(name="rb", bufs=3))
            p_ost = s_all.enter_context(tc.tile_pool(name="ost", bufs=4))
            ps_sc = s_all.enter_context(tc.tile_pool(name="ps_sc", bufs=3, space="PSUM"))
            ps_u = s_all.enter_context(tc.tile_pool(name="ps_u", bufs=2, space="PSUM"))

            def normalize(h, ic, up):
                """yT[h, ic] = U'[0:64] / rowsum."""
                poff = 64 * (h % 2)
                usb = p_usb.tile([HD, 512], f32, tag="usb", name="usb")
                nc.vector.tensor_copy(usb[:], up[0:HD, :])
                rs = p_rb.tile([1, 512], f32, tag="rs", name="rs")
                nc.vector.tensor_copy(rs[:], up[HD : HD + 1, :])
                rr = p_rb.tile([1, 512], f32, tag="rr", name="rr")
                nc.vector.reciprocal_approx_fast(rr[:], rs[:])
                rb = p_rb.tile([HD, 512], f32, tag="rb", name="rb")
                nc.gpsimd.partition_broadcast(rb[:], rr[0:1, :], channels=HD)
                nc.vector.tensor_mul(
                    yTn[ic][poff : poff + HD, h // 2, :], usb[:], rb[:]
                )

            def emit_proj(tns, cos):
                """out^T tiles for finished i-chunks; always-ready PE filler."""
                for co in cos:
                    psp = ps_sc.tile([128, 2, 512], f32, tag="sc", name="psp")
                    for ix, tn in enumerate(tns):
                        for db in range(DB):
                            nc.tensor.matmul(
                                psp[:, ix, :],
                                wp_sb[:, db, co * 128 : (co + 1) * 128],
                                yTn[tn][:, db, :],
                                start=(db == 0),
                                stop=(db == DB - 1),
                            )
                    ot = p_ost.tile([128, 2, 512], f32, tag="ot")
                    nc.vector.tensor_copy(ot[:], psp[:])
                    for ix, tn in enumerate(tns):
                        nc.sync.dma_start(
                            outT_ap[co * 128 : (co + 1) * 128, tn * 512 : (tn + 1) * 512],
                            ot[:, ix, :],
                        )

            for icp in range(NIC // 2):
                ics = [2 * icp, 2 * icp + 1]
                for h in range(HG):
                    poff = 64 * (h % 2)
                    co_q = h // 2
                    co_k = 4 + h // 2
                    ups = {
                        ic: ps_u.tile([HD + 1, 512], f32, tag="u", name=f"u_{ic}")
                        for ic in ics
                    }
                    # all (jt, ic) sub-tiles in jt-major order, packed in pairs
                    subs = [
                        (jt, ic)
                        for jt in range(4 * (ics[-1] + 1))
                        for ic in ics
                        if 4 * (ic + 1) > jt
                    ]
                    for g0 in range(0, len(subs), 2):
                        grp = subs[g0 : g0 + 2]
                        psg = ps_sc.tile([128, 2, 512], f32, tag="sc")
                        for ix, (jt, ic) in enumerate(grp):
                            m = jt % 4
                            diag = ic == jt // 4
                            lo = 128 * m if diag else 0
                            nc.tensor.matmul(
                                psg[:, ix, lo:512],
                                qkT[(co_k, jt // 4)][
                                    poff : poff + 64, m * 128 : (m + 1) * 128
                                ],
                                qkT[(co_q, ic)][poff : poff + 64, lo:512],
                                start=True,
                                stop=not diag,
                            )
                            if diag:  # -60000 above the diagonal -> exp == 0
                                nc.tensor.matmul(
                                    psg[:, ix, lo : lo + 128],
                                    negI[:],
                                    masks[:, m, lo : lo + 128],
                                    start=False,
                                    stop=True,
                                )
                        pt = p_p.tile([128, 2, 512], f16, tag="p")
                        nv = len(grp)
                        nc.scalar.activation(
                            pt[:, 0:nv, :], psg[:, 0:nv, :], Exp, scale=1.0 / np.sqrt(HD)
                        )
                        for ix, (jt, ic) in enumerate(grp):
                            m = jt % 4
                            diag = ic == jt // 4
                            lo = 128 * m if diag else 0
                            nc.tensor.matmul(
                                ups[ic][:, lo:512],
                                vv[jt][:, h, :],
                                pt[:, ix, lo:512],
                                start=(jt == 0),
                                stop=(jt == 4 * ic + 3),
                            )
                            if jt == 4 * ic + 3:
                                normalize(h, ic, ups[ic])
                    if icp > 0:
                        # previous icp's projection, one co per head: PE filler
                        emit_proj([2 * icp - 2, 2 * icp - 1], [h])
            emit_proj([NIC - 2, NIC - 1], range(CB))


def _build_program():
    nc = bacc.Bacc("TRN2", target_bir_lowering=False, debug=False, num_devices=N_CORES)
    aps = {
        "x": nc.dram_tensor("x", [T, C], f16, kind="ExternalInput").ap(),
        "wqk": nc.dram_tensor("wqk", [C, CB * 128], f16, kind="ExternalInput").ap(),
        "wv": nc.dram_tensor("wv", [C, CL], f16, kind="ExternalInput").ap(),
        "wp": nc.dram_tensor("wp", [CL, C], f16, kind="ExternalInput").ap(),
        "bqk": nc.dram_tensor("bqk", [CB, 128], f32, kind="ExternalInput").ap(),
        "masks": nc.dram_tensor("masks", [128, 4, 512], f16, kind="ExternalInput").ap(),
        "negI": nc.dram_tensor("negI", [128, 128], f16, kind="ExternalInput").ap(),
        "outT": nc.dram_tensor("outT", [C, T], f32, kind="ExternalOutput").ap(),
    }
    with tile.TileContext(nc) as tc:
        _emit(tc, aps)
    nc.compile()
    return nc


def get_program():
    if "nc" not in _PROG_CACHE:
        _PROG_CACHE["nc"] = _build_program()
    return _PROG_CACHE["nc"]


def _host_consts():
    j = np.arange(128)[:, None]
    i = np.arange(512)[None, :]
    masks = np.zeros((128, 4, 512), np.float16)
    for m in range(4):
        masks[:, m, :] = (j > i - 128 * m).astype(np.float16)  # 1 => mask out
    negI = (-60000.0 * np.eye(128)).astype(np.float16)
    return masks, negI


def make_in_maps(x, W_attn, b_attn, W_proj):
    """Build the 8 per-core input maps. Core index = 2*batch + head_group."""
    masks, negI = _host_consts()
    in_maps = []
    for core in range(N_CORES):
        b = core // 2
        g = core % 2
        wq = W_attn[:, g * CL : (g + 1) * CL]
        wk = W_attn[:, C + g * CL : C + (g + 1) * CL]
        wqk = np.concatenate([wq, wk], axis=1)  # [C, 1024], cols = co*128+q
        wv = W_attn[:, 2 * C + g * CL : 2 * C + (g + 1) * CL]
        bqk = np.concatenate(
            [b_attn[g * CL : (g + 1) * CL], b_attn[C + g * CL : C + (g + 1) * CL]]
        ).reshape(CB, 128)
        in_maps.append(
            {
                "x": np.ascontiguousarray(x[b]).astype(np.float16),
                "wqk": np.ascontiguousarray(wqk).astype(np.float16),
                "wv": np.ascontiguousarray(wv).astype(np.float16),
                "wp": np.ascontiguousarray(W_proj[g * CL : (g + 1) * CL, :]).astype(
                    np.float16
                ),
                "bqk": np.ascontiguousarray(bqk).astype(np.float32),
                "masks": masks,
                "negI": negI,
            }
        )
    return in_maps


def run(x, W_attn, b_attn, W_proj, b_proj, trace=False):
    nc = get_program()
    in_maps = make_in_maps(x, W_attn, b_attn, W_proj)
    res = bass_utils.run_bass_kernel_spmd(
        nc, in_maps, core_ids=list(range(N_CORES)), trace=trace
    )
    # combine: out[b] = sum_g outT_{2b+g}^T + (bv_g @ Wp_g summed) + b_proj
    corr = b_proj.astype(np.float64).copy()
    for g in range(2):
        bv_g = b_attn[2 * C + g * CL : 2 * C + (g + 1) * CL]
        corr += bv_g.astype(np.float64) @ W_proj[g * CL : (g + 1) * CL, :].astype(
            np.float64
        )
    out = np.empty((B, T, C), np.float32)
    for b in range(B):
        acc = (
            res.results[2 * b]["outT"].T.astype(np.float64)
            + res.results[2 * b + 1]["outT"].T.astype(np.float64)
            + corr
        )
        out[b] = acc.astype(np.float32)
    return out, res


def kernel(x, W_attn, b_attn, W_proj, b_proj):
    x = np.asarray(x, np.float32)
    W_attn = np.asarray(W_attn, np.float32)
    b_attn = np.asarray(b_attn, np.float32)
    W_proj = np.asarray(W_proj, np.float32)
    b_proj = np.asarray(b_proj, np.float32)
    out, _ = run(x, W_attn, b_attn, W_proj, b_proj)
    return out



# revision 3
# speedup vs baseline: 1.0499x; 1.0499x over previous
"""Causal self-attention (B=4, T=2048, C=1024, H=16) on 8 Trainium2 NeuronCores.

Core index = 2*batch + head_group: each core owns one batch element and 8 of
the 16 heads (tensor-parallel split of c_attn output dim / c_proj input dim).
Each core emits a partial projection out^T [C, T]; the host sums the two
head-group partials per batch and adds the bias terms.

v2 (vs v1 baseline 358us):
  * Head-PAIR processing: even local head lives on SBUF partitions 0-63,
    odd head on 64-127. The two K=64 scores matmuls of a pair are emitted
    back-to-back with inferred tile_position (0,0)/(64,0) -> they run
    CONCURRENTLY in the PE array (row-group tiling), and their LDWEIGHTS
    cross-hide against the other row group's matmul.
  * No negI mask matmuls: causal masking is a post-exp DVE multiply of the
    128-wide diagonal block only (PE -21us, DVE +16us).
  * qkv projection (phase B), attention (phase C) and out-projection are
    interleaved in emission order so the Tile scheduler can fill PE stalls
    (C is ACT-exp-bound in bursts) with independent B/proj matmuls.
  * x DMA-transposes split across the two HWDGE queues (sync+scalar) and
    issued first: PE starts ~6us in instead of ~24us.
  * normalize reads U'/rowsum directly from PSUM (no staging copies).

fp16 datapath (fp32 PSUM accumulation everywhere, fp32 softmax denominator).

Per-core pipeline per head pair u (heads 2u, 2u+1), per 512-wide i-chunk ic:
  S^T[j, i] = k_h^T q_h  for both heads -> one psum tile [128, 2, 512]
  P = exp(S^T / 8)       (one ACT op per (jt, ic) sub, both heads)
  diagonal j-tile: P *= keep (DVE, precomputed lower-tri mask)
  U'^T [65, i] (+)= [v_h|1]^T P_h^T  over j-tiles (ones col => rowsum row 64)
  yT[u-block, i] = U'[0:64] * bcast(1/rowsum)
out^T = W_p^T yT -> fp32 psum -> DVE copy -> DMA
"""

import numpy as np

import concourse.bass as bass
import concourse.mybir as mybir
import concourse.tile as tile
from concourse import bacc, bass_utils

B, T, C, H = 4, 2048, 1024, 16
HD = C // H          # 64 head dim
N_CORES = 8
HG = H // 2          # 8 heads per core
CL = HG * HD         # 512 local width of q/k/v
TT = T // 128        # 16 t-tiles
CB = C // 128        # 8 c-tiles
DB = CL // 128       # 4 local-hd tiles
NIC = T // 512       # i-chunks (4)
NP = HG // 2         # head pairs per core (4)

f32 = mybir.dt.float32
f16 = mybir.dt.float16

_PROG_CACHE = {}


def _emit(tc, aps):
    nc = tc.nc
    Exp = mybir.ActivationFunctionType.Exp
    Copy = mybir.ActivationFunctionType.Copy

    x_ap = aps["x"]
    wqk_ap = aps["wqk"]
    wv_ap = aps["wv"]
    wp_ap = aps["wp"]
    bqk_ap = aps["bqk"]
    keep2_ap = aps["keep2"]
    outT_ap = aps["outT"]

    from contextlib import ExitStack

    with ExitStack() as outer:
        const = outer.enter_context(tc.tile_pool(name="const", bufs=1))
        p_xT = outer.enter_context(tc.tile_pool(name="xT", bufs=1))
        p_qkT = outer.enter_context(tc.tile_pool(name="qkT", bufs=1))
        p_v = outer.enter_context(tc.tile_pool(name="vv", bufs=1))
        p_yT = outer.enter_context(tc.tile_pool(name="yT", bufs=1))
        p_w = outer.enter_context(tc.tile_pool(name="wsb", bufs=1))
        p_p = outer.enter_context(tc.tile_pool(name="pp", bufs=8))
        p_rb = outer.enter_context(tc.tile_pool(name="rb", bufs=3))
        p_ost = outer.enter_context(tc.tile_pool(name="ost", bufs=4))
        ps_b = outer.enter_context(tc.tile_pool(name="ps_b", bufs=2, space="PSUM"))
        ps_sc = outer.enter_context(tc.tile_pool(name="ps_sc", bufs=2, space="PSUM"))
        ps_u = outer.enter_context(tc.tile_pool(name="ps_u", bufs=2, space="PSUM"))

        # ---- DMAs: x transposes first (PE critical path), split across the
        # two HWDGE queues; weights interleave behind them.
        xT = p_xT.tile([128, CB, T], f16)
        wqk_sb = p_w.tile([128, CB, CB * 128], f16)  # [c-part, cb, co*128+q]
        nc.sync.dma_start_transpose(xT[:, :, 0:512], x_ap[0:512, :])
        nc.scalar.dma_start(wqk_sb[:], wqk_ap.rearrange("(cb p) n -> p cb n", p=128))
        nc.scalar.dma_start_transpose(xT[:, :, 512:1024], x_ap[512:1024, :])
        wv_sb = p_w.tile([128, CB, CL], f16)
        nc.sync.dma_start(wv_sb[:], wv_ap.rearrange("(cb p) n -> p cb n", p=128))
        nc.sync.dma_start_transpose(xT[:, :, 1024:1536], x_ap[1024:1536, :])
        nc.scalar.dma_start_transpose(xT[:, :, 1536:2048], x_ap[1536:2048, :])
        wp_sb = p_w.tile([128, DB, C], f16)
        nc.sync.dma_start(wp_sb[:], wp_ap.rearrange("(db p) c -> p db c", p=128))
        keep2 = const.tile([128, 2, 128], f16)   # keep[j, ix, i] = (j <= i)
        nc.gpsimd.dma_start(keep2[:], keep2_ap)
        bqk = const.tile([128, CB], f32)
        nc.gpsimd.dma_start(bqk[:], bqk_ap.rearrange("co p -> p co"))

        # per-(co, tn) qkT tiles, per-jt v' tiles, per-tn yT tiles
        qkT = {}
        for co in range(CB):
            for tn in range(NIC):
                qkT[(co, tn)] = p_qkT.tile(
                    [128, 512], f16, tag=f"qkT_{co}_{tn}", name=f"qkT_{co}_{tn}"
                )
        vv = {}
        for jt in range(TT):
            vv[jt] = p_v.tile([128, HG, HD + 1], f16, tag=f"vv_{jt}", name=f"vv_{jt}")
            nc.vector.memset(vv[jt][:, :, HD : HD + 1], 1.0)
        yTn = {}
        for tn in range(NIC):
            yTn[tn] = p_yT.tile([128, DB, 512], f16, tag=f"yT_{tn}", name=f"yT_{tn}")

        def emit_qkT_group(co, tn):
            ps = ps_b.tile([128, 512], f32, tag="bps")
            for cb in range(CB):
                nc.tensor.matmul(
                    ps[:],
                    wqk_sb[:, cb, co * 128 : (co + 1) * 128],
                    xT[:, cb, tn * 512 : (tn + 1) * 512],
                    start=(cb == 0),
                    stop=(cb == CB - 1),
                )
            nc.vector.tensor_scalar_add(qkT[(co, tn)][:], ps[:], bqk[:, co : co + 1])

        def emit_vv_group(tt):
            ps = ps_b.tile([128, CL], f32, tag="bps")
            for cb in range(CB):
                nc.tensor.matmul(
                    ps[:],
                    xT[:, cb, tt * 128 : (tt + 1) * 128],
                    wv_sb[:, cb, :],
                    start=(cb == 0),
                    stop=(cb == CB - 1),
                )
            nc.scalar.activation(
                vv[tt][:, :, 0:HD],
                ps.rearrange("p (h d) -> p h d", d=HD),
                Copy,
            )

        def emit_pair(u, ic):
            """Attention for head pair (2u, 2u+1) over i-chunk ic."""
            co_q = u
            co_k = 4 + u
            nj = 4 * ic + 4
            ups_e = ps_u.tile([HD + 1, 512], f32, tag="u", name=f"ue_{u}_{ic}")
            ups_o = ps_u.tile([HD + 1, 512], f32, tag="u", name=f"uo_{u}_{ic}")
            for jt in range(nj):
                m = jt % 4
                diag = ic == jt // 4
                lo = 128 * m if diag else 0
                psg = ps_sc.tile([128, 2, 512], f32, tag="sc")
                for ix in range(2):
                    r0 = 64 * ix
                    nc.tensor.matmul(
                        psg[:, ix, lo:512],
                        qkT[(co_k, jt // 4)][r0 : r0 + 64, m * 128 : (m + 1) * 128],
                        qkT[(co_q, ic)][r0 : r0 + 64, lo:512],
                        start=True,
                        stop=True,
                    )
                pt = p_p.tile([128, 2, 512], f16, tag="p")
                nc.scalar.activation(
                    pt[:, 0:2, lo:512], psg[:, 0:2, lo:512], Exp, scale=1.0 / np.sqrt(HD)
                )
                if diag:  # zero strictly-upper part of the diagonal block
                    nc.vector.tensor_mul(
                        pt[:, 0:2, lo : lo + 128],
                        pt[:, 0:2, lo : lo + 128],
                        keep2[:, 0:2, :],
                    )
                for ix, ups in ((0, ups_e), (1, ups_o)):
                    nc.tensor.matmul(
                        ups[:, lo:512],
                        vv[jt][:, 2 * u + ix, :],
                        pt[:, ix, lo:512],
                        start=(jt == 0),
                        stop=(jt == nj - 1),
                    )
            for ups, r0 in ((ups_e, 0), (ups_o, 64)):
                rs = p_rb.tile([1, 512], f32, tag="rs", name="rs")
                nc.vector.tensor_copy(rs[:], ups[HD : HD + 1, :])
                rr = p_rb.tile([1, 512], f32, tag="rr", name="rr")
                nc.vector.reciprocal_approx_fast(rr[:], rs[:])
                rb = p_rb.tile([HD, 512], f32, tag="rb", name="rb")
                nc.gpsimd.partition_broadcast(rb[:], rr[0:1, :], channels=HD)
                nc.vector.tensor_mul(
                    yTn[ic][r0 : r0 + HD, u, :], ups[0:HD, :], rb[:]
                )

        def emit_proj(tn, cos):
            for co in cos:
                psp = ps_b.tile([128, 512], f32, tag="bps")
                for db in range(DB):
                    nc.tensor.matmul(
                        psp[:],
                        wp_sb[:, db, co * 128 : (co + 1) * 128],
                        yTn[tn][:, db, :],
                        start=(db == 0),
                        stop=(db == DB - 1),
                    )
                ot = p_ost.tile([128, 512], f32, tag="ot")
                nc.vector.tensor_copy(ot[:], psp[:])
                nc.sync.dma_start(
                    outT_ap[co * 128 : (co + 1) * 128, tn * 512 : (tn + 1) * 512],
                    ot[:],
                )

        # ---- B(0) up front; then C(ic) with B(ic+1)/proj(ic-1) interleaved
        for co in range(CB):
            emit_qkT_group(co, 0)
        for tt in range(4):
            emit_vv_group(tt)

        for ic in range(NIC):
            for u in range(NP):
                emit_pair(u, ic)
                if ic < NIC - 1:
                    emit_qkT_group(2 * u, ic + 1)
                    emit_qkT_group(2 * u + 1, ic + 1)
                    emit_vv_group(4 * (ic + 1) + u)
                if ic >= 1:
                    emit_proj(ic - 1, [2 * u, 2 * u + 1])
        emit_proj(NIC - 1, range(CB))


def _build_program():
    nc = bacc.Bacc("TRN2", target_bir_lowering=False, debug=False, num_devices=N_CORES)
    aps = {
        "x": nc.dram_tensor("x", [T, C], f16, kind="ExternalInput").ap(),
        "wqk": nc.dram_tensor("wqk", [C, CB * 128], f16, kind="ExternalInput").ap(),
        "wv": nc.dram_tensor("wv", [C, CL], f16, kind="ExternalInput").ap(),
        "wp": nc.dram_tensor("wp", [CL, C], f16, kind="ExternalInput").ap(),
        "bqk": nc.dram_tensor("bqk", [CB, 128], f32, kind="ExternalInput").ap(),
        "keep2": nc.dram_tensor("keep2", [128, 2, 128], f16, kind="ExternalInput").ap(),
        "outT": nc.dram_tensor("outT", [C, T], f32, kind="ExternalOutput").ap(),
    }
    with tile.TileContext(nc) as tc:
        _emit(tc, aps)
    nc.compile()
    return nc


def get_program():
    if "nc" not in _PROG_CACHE:
        _PROG_CACHE["nc"] = _build_program()
    return _PROG_CACHE["nc"]


def _host_consts():
    j = np.arange(128)[:, None]
    i = np.arange(128)[None, :]
    keep = (j <= i).astype(np.float16)          # 1 => keep
    keep2 = np.stack([keep, keep], axis=1)      # [128, 2, 128]
    return np.ascontiguousarray(keep2)


def make_in_maps(x, W_attn, b_attn, W_proj):
    """Build the 8 per-core input maps. Core index = 2*batch + head_group."""
    keep2 = _host_consts()
    in_maps = []
    for core in range(N_CORES):
        b = core // 2
        g = core % 2
        wq = W_attn[:, g * CL : (g + 1) * CL]
        wk = W_attn[:, C + g * CL : C + (g + 1) * CL]
        wqk = np.concatenate([wq, wk], axis=1)  # [C, 1024], cols = co*128+q
        wv = W_attn[:, 2 * C + g * CL : 2 * C + (g + 1) * CL]
        bqk = np.concatenate(
            [b_attn[g * CL : (g + 1) * CL], b_attn[C + g * CL : C + (g + 1) * CL]]
        ).reshape(CB, 128)
        in_maps.append(
            {
                "x": np.ascontiguousarray(x[b]).astype(np.float16),
                "wqk": np.ascontiguousarray(wqk).astype(np.float16),
                "wv": np.ascontiguousarray(wv).astype(np.float16),
                "wp": np.ascontiguousarray(W_proj[g * CL : (g + 1) * CL, :]).astype(
                    np.float16
                ),
                "bqk": np.ascontiguousarray(bqk).astype(np.float32),
                "keep2": keep2,
            }
        )
    return in_maps


def run(x, W_attn, b_attn, W_proj, b_proj, trace=False):
    nc = get_program()
    in_maps = make_in_maps(x, W_attn, b_attn, W_proj)
    res = bass_utils.run_bass_kernel_spmd(
        nc, in_maps, core_ids=list(range(N_CORES)), trace=trace
    )
    # combine: out[b] = sum_g outT_{2b+g}^T + (bv_g @ Wp_g summed) + b_proj
    corr = b_proj.astype(np.float64).copy()
    for g in range(2):
        bv_g = b_attn[2 * C + g * CL : 2 * C + (g + 1) * CL]
        corr += bv_g.astype(np.float64) @ W_proj[g * CL : (g + 1) * CL, :].astype(
            np.float64
        )
    out = np.empty((B, T, C), np.float32)
    for b in range(B):
        acc = (
            res.results[2 * b]["outT"].T.astype(np.float64)
            + res.results[2 * b + 1]["outT"].T.astype(np.float64)
            + corr
        )
        out[b] = acc.astype(np.float32)
    return out, res


def kernel(x, W_attn, b_attn, W_proj, b_proj):
    x = np.asarray(x, np.float32)
    W_attn = np.asarray(W_attn, np.float32)
    b_attn = np.asarray(b_attn, np.float32)
    W_proj = np.asarray(W_proj, np.float32)
    b_proj = np.asarray(b_proj, np.float32)
    out, _ = run(x, W_attn, b_attn, W_proj, b_proj)
    return out


# revision 5
# speedup vs baseline: 1.2028x; 1.1457x over previous
"""Causal self-attention (B=4, T=2048, C=1024, H=16) on 8 Trainium2 NeuronCores.

Core index = 2*batch + head_group: each core owns one batch element and 8 of
the 16 heads (tensor-parallel split of c_attn output dim / c_proj input dim).
Each core emits a partial projection out^T [C, T]; the host sums the two
head-group partials per batch and adds the bias terms.

v2 (vs v1 baseline 358us):
  * Head-PAIR processing: even local head lives on SBUF partitions 0-63,
    odd head on 64-127. The two K=64 scores matmuls of a pair are emitted
    back-to-back with inferred tile_position (0,0)/(64,0) -> they run
    CONCURRENTLY in the PE array (row-group tiling), and their LDWEIGHTS
    cross-hide against the other row group's matmul.
  * No negI mask matmuls: causal masking is a post-exp DVE multiply of the
    128-wide diagonal block only (PE -21us, DVE +16us).
  * qkv projection (phase B), attention (phase C) and out-projection are
    interleaved in emission order so the Tile scheduler can fill PE stalls
    (C is ACT-exp-bound in bursts) with independent B/proj matmuls.
  * x DMA-transposes split across the two HWDGE queues (sync+scalar) and
    issued first: PE starts ~6us in instead of ~24us.
  * normalize reads U'/rowsum directly from PSUM (no staging copies).

fp16 datapath (fp32 PSUM accumulation everywhere, fp32 softmax denominator).

Per-core pipeline per head pair u (heads 2u, 2u+1), per 512-wide i-chunk ic:
  S^T[j, i] = k_h^T q_h  for both heads -> one psum tile [128, 2, 512]
  P = exp(S^T / 8)       (one ACT op per (jt, ic) sub, both heads)
  diagonal j-tile: P *= keep (DVE, precomputed lower-tri mask)
  U'^T [65, i] (+)= [v_h|1]^T P_h^T  over j-tiles (ones col => rowsum row 64)
  yT[u-block, i] = U'[0:64] * bcast(1/rowsum)
out^T = W_p^T yT -> fp32 psum -> DVE copy -> DMA
"""

import numpy as np

import concourse.bass as bass
import concourse.mybir as mybir
import concourse.tile as tile
from concourse import bacc, bass_utils

B, T, C, H = 4, 2048, 1024, 16
HD = C // H          # 64 head dim
N_CORES = 8
HG = H // 2          # 8 heads per core
CL = HG * HD         # 512 local width of q/k/v
TT = T // 128        # 16 t-tiles
CB = C // 128        # 8 c-tiles
DB = CL // 128       # 4 local-hd tiles
NIC = T // 512       # i-chunks (4)
NP = HG // 2         # head pairs per core (4)

f32 = mybir.dt.float32
f16 = mybir.dt.float16

_PROG_CACHE = {}


def _emit(tc, aps):
    nc = tc.nc
    Exp = mybir.ActivationFunctionType.Exp
    Copy = mybir.ActivationFunctionType.Copy

    x_ap = aps["x"]
    wqk_ap = aps["wqk"]
    wv_ap = aps["wv"]
    wp_ap = aps["wp"]
    bqk_ap = aps["bqk"]
    keep2_ap = aps["keep2"]
    outT_ap = aps["outT"]

    from contextlib import ExitStack

    with ExitStack() as outer:
        const = outer.enter_context(tc.tile_pool(name="const", bufs=1))
        p_xT = outer.enter_context(tc.tile_pool(name="xT", bufs=1))
        p_qkT = outer.enter_context(tc.tile_pool(name="qkT", bufs=1))
        p_v = outer.enter_context(tc.tile_pool(name="vv", bufs=1))
        p_yT = outer.enter_context(tc.tile_pool(name="yT", bufs=1))
        p_w = outer.enter_context(tc.tile_pool(name="wsb", bufs=1))
        p_p = outer.enter_context(tc.tile_pool(name="pp", bufs=8))
        p_rb = outer.enter_context(tc.tile_pool(name="rb", bufs=3))
        p_ost = outer.enter_context(tc.tile_pool(name="ost", bufs=4))
        ps_b = outer.enter_context(tc.tile_pool(name="ps_b", bufs=2, space="PSUM"))
        ps_sc = outer.enter_context(tc.tile_pool(name="ps_sc", bufs=2, space="PSUM"))
        ps_u = outer.enter_context(tc.tile_pool(name="ps_u", bufs=2, space="PSUM"))

        # ---- DMAs: x transposes are the PE critical path -> they own both
        # HWDGE queues (sync+scalar), with t-chunk 0 split across the two so
        # its descriptor GENERATION (~6us each) runs in parallel. All weight
        # loads go through the gpsimd SWDGE queue (cheap generation).
        xT = p_xT.tile([128, CB, T], f16)
        nc.sync.dma_start_transpose(xT[:, :, 0:256], x_ap[0:256, :])
        nc.scalar.dma_start_transpose(xT[:, :, 256:512], x_ap[256:512, :])
        nc.sync.dma_start_transpose(xT[:, :, 1024:1536], x_ap[1024:1536, :])
        nc.scalar.dma_start_transpose(xT[:, :, 512:1024], x_ap[512:1024, :])
        nc.scalar.dma_start_transpose(xT[:, :, 1536:2048], x_ap[1536:2048, :])
        wqk_sb = p_w.tile([128, CB, CB * 128], f16)  # [c-part, cb, co*128+q]
        nc.gpsimd.dma_start(wqk_sb[:], wqk_ap.rearrange("(cb p) n -> p cb n", p=128))
        wv_sb = p_w.tile([128, CB, CL], f16)
        nc.gpsimd.dma_start(wv_sb[:], wv_ap.rearrange("(cb p) n -> p cb n", p=128))
        wp_sb = p_w.tile([128, DB, C], f16)
        nc.gpsimd.dma_start(wp_sb[:], wp_ap.rearrange("(db p) c -> p db c", p=128))
        keep2 = const.tile([128, 2, 128], f16)   # keep[j, ix, i] = (j <= i)
        nc.gpsimd.dma_start(keep2[:], keep2_ap)
        bqk = const.tile([128, CB], f32)
        nc.gpsimd.dma_start(bqk[:], bqk_ap.rearrange("co p -> p co"))

        # ---- PE warm-up: dep-free matmuls on a zeroed tile keep the PE busy
        # from t~5us so HAM un-throttles to 2.4 GHz before real work arrives,
        # and the array never sits cold waiting for the first x chunk.
        warm = const.tile([128, 512], f16)
        nc.vector.memset(warm[:], 0.0)
        wups = ps_u.tile([HD + 1, 512], f32, tag="u", name="wups")
        for _ in range(18):
            nc.tensor.matmul(
                wups[:], warm[:, 0 : HD + 1], warm[:], start=True, stop=True
            )

        # per-(co, tn) qkT tiles, per-jt v' tiles, per-tn yT tiles
        qkT = {}
        for co in range(CB):
            for tn in range(NIC):
                qkT[(co, tn)] = p_qkT.tile(
                    [128, 512], f16, tag=f"qkT_{co}_{tn}", name=f"qkT_{co}_{tn}"
                )
        vv = {}
        for jt in range(TT):
            vv[jt] = p_v.tile([128, HG, HD + 1], f16, tag=f"vv_{jt}", name=f"vv_{jt}")
            nc.vector.memset(vv[jt][:, :, HD : HD + 1], 1.0)
        yTn = {}
        for tn in range(NIC):
            yTn[tn] = p_yT.tile([128, DB, 512], f16, tag=f"yT_{tn}", name=f"yT_{tn}")

        def emit_qkT_group(co, tn):
            ps = ps_b.tile([128, 512], f32, tag="bps")
            for cb in range(CB):
                nc.tensor.matmul(
                    ps[:],
                    wqk_sb[:, cb, co * 128 : (co + 1) * 128],
                    xT[:, cb, tn * 512 : (tn + 1) * 512],
                    start=(cb == 0),
                    stop=(cb == CB - 1),
                )
            nc.vector.tensor_scalar_add(qkT[(co, tn)][:], ps[:], bqk[:, co : co + 1])

        def emit_vv_group(tt):
            ps = ps_b.tile([128, CL], f32, tag="bps")
            for cb in range(CB):
                nc.tensor.matmul(
                    ps[:],
                    xT[:, cb, tt * 128 : (tt + 1) * 128],
                    wv_sb[:, cb, :],
                    start=(cb == 0),
                    stop=(cb == CB - 1),
                )
            nc.scalar.activation(
                vv[tt][:, :, 0:HD],
                ps.rearrange("p (h d) -> p h d", d=HD),
                Copy,
            )

        def emit_pair(u, ic):
            """Attention for head pair (2u, 2u+1) over i-chunk ic."""
            co_q = u
            co_k = 4 + u
            nj = 4 * ic + 4
            ups_e = ps_u.tile([HD + 1, 512], f32, tag="u", name=f"ue_{u}_{ic}")
            ups_o = ps_u.tile([HD + 1, 512], f32, tag="u", name=f"uo_{u}_{ic}")
            for jt in range(nj):
                m = jt % 4
                diag = ic == jt // 4
                lo = 128 * m if diag else 0
                psg = ps_sc.tile([128, 2, 512], f32, tag="sc")
                for ix in range(2):
                    r0 = 64 * ix
                    nc.tensor.matmul(
                        psg[:, ix, lo:512],
                        qkT[(co_k, jt // 4)][r0 : r0 + 64, m * 128 : (m + 1) * 128],
                        qkT[(co_q, ic)][r0 : r0 + 64, lo:512],
                        start=True,
                        stop=True,
                    )
                pt = p_p.tile([128, 2, 512], f16, tag="p")
                nc.scalar.activation(
                    pt[:, 0:2, lo:512], psg[:, 0:2, lo:512], Exp, scale=1.0 / np.sqrt(HD)
                )
                if diag:  # zero strictly-upper part of the diagonal block
                    nc.vector.tensor_mul(
                        pt[:, 0:2, lo : lo + 128],
                        pt[:, 0:2, lo : lo + 128],
                        keep2[:, 0:2, :],
                    )
                for ix, ups in ((0, ups_e), (1, ups_o)):
                    nc.tensor.matmul(
                        ups[:, lo:512],
                        vv[jt][:, 2 * u + ix, :],
                        pt[:, ix, lo:512],
                        start=(jt == 0),
                        stop=(jt == nj - 1),
                    )
            for ups, r0 in ((ups_e, 0), (ups_o, 64)):
                rs = p_rb.tile([1, 512], f32, tag="rs", name="rs")
                nc.vector.tensor_copy(rs[:], ups[HD : HD + 1, :])
                rr = p_rb.tile([1, 512], f32, tag="rr", name="rr")
                nc.vector.reciprocal_approx_fast(rr[:], rs[:])
                rb = p_rb.tile([HD, 512], f32, tag="rb", name="rb")
                nc.gpsimd.partition_broadcast(rb[:], rr[0:1, :], channels=HD)
                nc.vector.tensor_mul(
                    yTn[ic][r0 : r0 + HD, u, :], ups[0:HD, :], rb[:]
                )

        def emit_proj(tn, cos):
            for co in cos:
                psp = ps_b.tile([128, 512], f32, tag="bps")
                for db in range(DB):
                    nc.tensor.matmul(
                        psp[:],
                        wp_sb[:, db, co * 128 : (co + 1) * 128],
                        yTn[tn][:, db, :],
                        start=(db == 0),
                        stop=(db == DB - 1),
                    )
                ot = p_ost.tile([128, 512], f32, tag="ot")
                nc.vector.tensor_copy(ot[:], psp[:])
                nc.sync.dma_start(
                    outT_ap[co * 128 : (co + 1) * 128, tn * 512 : (tn + 1) * 512],
                    ot[:],
                )

        # ---- B(0) up front; then C(ic) with B(ic+1)/proj(ic-1) interleaved
        for co in range(CB):
            emit_qkT_group(co, 0)
        for tt in range(4):
            emit_vv_group(tt)

        for ic in range(NIC):
            for u in range(NP):
                emit_pair(u, ic)
                if ic < NIC - 1:
                    emit_qkT_group(2 * u, ic + 1)
                    emit_qkT_group(2 * u + 1, ic + 1)
                    emit_vv_group(4 * (ic + 1) + u)
                if ic >= 1 and u < 2:
                    emit_proj(ic - 1, [2 * u, 2 * u + 1])
            # second half of the previous chunk's projection lands after the
            # last pair: dep-free PE filler over the final normalize chain
            if ic >= 1:
                emit_proj(ic - 1, [4, 5, 6, 7])
        emit_proj(NIC - 1, range(CB))


def _build_program():
    nc = bacc.Bacc("TRN2", target_bir_lowering=False, debug=False, num_devices=N_CORES)
    aps = {
        "x": nc.dram_tensor("x", [T, C], f16, kind="ExternalInput").ap(),
        "wqk": nc.dram_tensor("wqk", [C, CB * 128], f16, kind="ExternalInput").ap(),
        "wv": nc.dram_tensor("wv", [C, CL], f16, kind="ExternalInput").ap(),
        "wp": nc.dram_tensor("wp", [CL, C], f16, kind="ExternalInput").ap(),
        "bqk": nc.dram_tensor("bqk", [CB, 128], f32, kind="ExternalInput").ap(),
        "keep2": nc.dram_tensor("keep2", [128, 2, 128], f16, kind="ExternalInput").ap(),
        "outT": nc.dram_tensor("outT", [C, T], f32, kind="ExternalOutput").ap(),
    }
    with tile.TileContext(nc) as tc:
        _emit(tc, aps)
    nc.compile()
    return nc


def get_program():
    if "nc" not in _PROG_CACHE:
        _PROG_CACHE["nc"] = _build_program()
    return _PROG_CACHE["nc"]


def _host_consts():
    j = np.arange(128)[:, None]
    i = np.arange(128)[None, :]
    keep = (j <= i).astype(np.float16)          # 1 => keep
    keep2 = np.stack([keep, keep], axis=1)      # [128, 2, 128]
    return np.ascontiguousarray(keep2)


def make_in_maps(x, W_attn, b_attn, W_proj):
    """Build the 8 per-core input maps. Core index = 2*batch + head_group."""
    keep2 = _host_consts()
    in_maps = []
    for core in range(N_CORES):
        b = core // 2
        g = core % 2
        wq = W_attn[:, g * CL : (g + 1) * CL]
        wk = W_attn[:, C + g * CL : C + (g + 1) * CL]
        wqk = np.concatenate([wq, wk], axis=1)  # [C, 1024], cols = co*128+q
        wv = W_attn[:, 2 * C + g * CL : 2 * C + (g + 1) * CL]
        bqk = np.concatenate(
            [b_attn[g * CL : (g + 1) * CL], b_attn[C + g * CL : C + (g + 1) * CL]]
        ).reshape(CB, 128)
        in_maps.append(
            {
                "x": np.ascontiguousarray(x[b]).astype(np.float16),
                "wqk": np.ascontiguousarray(wqk).astype(np.float16),
                "wv": np.ascontiguousarray(wv).astype(np.float16),
                "wp": np.ascontiguousarray(W_proj[g * CL : (g + 1) * CL, :]).astype(
                    np.float16
                ),
                "bqk": np.ascontiguousarray(bqk).astype(np.float32),
                "keep2": keep2,
            }
        )
    return in_maps


def run(x, W_attn, b_attn, W_proj, b_proj, trace=False):
    nc = get_program()
    in_maps = make_in_maps(x, W_attn, b_attn, W_proj)
    res = bass_utils.run_bass_kernel_spmd(
        nc, in_maps, core_ids=list(range(N_CORES)), trace=trace
    )
    # combine: out[b] = sum_g outT_{2b+g}^T + (bv_g @ Wp_g summed) + b_proj
    corr = b_proj.astype(np.float64).copy()
    for g in range(2):
        bv_g = b_attn[2 * C + g * CL : 2 * C + (g + 1) * CL]
        corr += bv_g.astype(np.float64) @ W_proj[g * CL : (g + 1) * CL, :].astype(
            np.float64
        )
    out = np.empty((B, T, C), np.float32)
    for b in range(B):
        acc = (
            res.results[2 * b]["outT"].T.astype(np.float64)
            + res.results[2 * b + 1]["outT"].T.astype(np.float64)
            + corr
        )
        out[b] = acc.astype(np.float32)
    return out, res


def kernel(x, W_attn, b_attn, W_proj, b_proj):
    x = np.asarray(x, np.float32)
    W_attn = np.asarray(W_attn, np.float32)
    b_attn = np.asarray(b_attn, np.float32)
    W_proj = np.asarray(W_proj, np.float32)
    b_proj = np.asarray(b_proj, np.float32)
    out, _ = run(x, W_attn, b_attn, W_proj, b_proj)
    return out


# revision 7
# speedup vs baseline: 1.2300x; 1.0226x over previous
"""Causal self-attention (B=4, T=2048, C=1024, H=16) on 8 Trainium2 NeuronCores.

Core index = 2*batch + head_group: each core owns one batch element and 8 of
the 16 heads (tensor-parallel split of c_attn output dim / c_proj input dim).
Each core emits a partial projection out^T [C, T]; the host sums the two
head-group partials per batch and adds the bias terms.

v2 (vs v1 baseline 358us):
  * Head-PAIR processing: even local head lives on SBUF partitions 0-63,
    odd head on 64-127. The two K=64 scores matmuls of a pair are emitted
    back-to-back with inferred tile_position (0,0)/(64,0) -> they run
    CONCURRENTLY in the PE array (row-group tiling), and their LDWEIGHTS
    cross-hide against the other row group's matmul.
  * No negI mask matmuls: causal masking is a post-exp DVE multiply of the
    128-wide diagonal block only (PE -21us, DVE +16us).
  * qkv projection (phase B), attention (phase C) and out-projection are
    interleaved in emission order so the Tile scheduler can fill PE stalls
    (C is ACT-exp-bound in bursts) with independent B/proj matmuls.
  * x DMA-transposes split across the two HWDGE queues (sync+scalar) and
    issued first: PE starts ~6us in instead of ~24us.
  * normalize reads U'/rowsum directly from PSUM (no staging copies).

fp16 datapath (fp32 PSUM accumulation everywhere, fp32 softmax denominator).

Per-core pipeline per head pair u (heads 2u, 2u+1), per 512-wide i-chunk ic:
  S^T[j, i] = k_h^T q_h  for both heads -> one psum tile [128, 2, 512]
  P = exp(S^T / 8)       (one ACT op per (jt, ic) sub, both heads)
  diagonal j-tile: P *= keep (DVE, precomputed lower-tri mask)
  U'^T [65, i] (+)= [v_h|1]^T P_h^T  over j-tiles (ones col => rowsum row 64)
  yT[u-block, i] = U'[0:64] * bcast(1/rowsum)
out^T = W_p^T yT -> fp32 psum -> DVE copy -> DMA
"""

import numpy as np

import concourse.bass as bass
import concourse.mybir as mybir
import concourse.tile as tile
from concourse import bacc, bass_utils

B, T, C, H = 4, 2048, 1024, 16
HD = C // H          # 64 head dim
N_CORES = 8
HG = H // 2          # 8 heads per core
CL = HG * HD         # 512 local width of q/k/v
TT = T // 128        # 16 t-tiles
CB = C // 128        # 8 c-tiles
DB = CL // 128       # 4 local-hd tiles
NIC = T // 512       # i-chunks (4)
NP = HG // 2         # head pairs per core (4)

f32 = mybir.dt.float32
f16 = mybir.dt.float16

_PROG_CACHE = {}


def _emit(tc, aps):
    nc = tc.nc
    Exp = mybir.ActivationFunctionType.Exp
    Copy = mybir.ActivationFunctionType.Copy

    x_ap = aps["x"]
    wqk_ap = aps["wqk"]
    wv_ap = aps["wv"]
    wp_ap = aps["wp"]
    bqk_ap = aps["bqk"]
    keep2_ap = aps["keep2"]
    outT_ap = aps["outT"]

    from contextlib import ExitStack

    with ExitStack() as outer:
        const = outer.enter_context(tc.tile_pool(name="const", bufs=1))
        p_xT = outer.enter_context(tc.tile_pool(name="xT", bufs=1))
        p_qkT = outer.enter_context(tc.tile_pool(name="qkT", bufs=1))
        p_v = outer.enter_context(tc.tile_pool(name="vv", bufs=1))
        p_yT = outer.enter_context(tc.tile_pool(name="yT", bufs=1))
        p_w = outer.enter_context(tc.tile_pool(name="wsb", bufs=1))
        p_p = outer.enter_context(tc.tile_pool(name="pp", bufs=8))
        p_rb = outer.enter_context(tc.tile_pool(name="rb", bufs=3))
        p_ost = outer.enter_context(tc.tile_pool(name="ost", bufs=4))
        ps_b = outer.enter_context(tc.tile_pool(name="ps_b", bufs=2, space="PSUM"))
        ps_sc = outer.enter_context(tc.tile_pool(name="ps_sc", bufs=2, space="PSUM"))
        ps_u = outer.enter_context(tc.tile_pool(name="ps_u", bufs=2, space="PSUM"))

        # ---- DMAs: x transposes are the PE critical path -> they own both
        # HWDGE queues (sync+scalar), with t-chunk 0 split across the two so
        # its descriptor GENERATION (~6us each) runs in parallel. All weight
        # loads go through the gpsimd SWDGE queue (cheap generation).
        # XBAR transposes serialize across queues -> all on sync, ascending,
        # 256-row pieces so the first B(0) dependency lands ~10us.
        xT = p_xT.tile([128, CB, T], f16)
        for h in range(8):
            nc.sync.dma_start_transpose(
                xT[:, :, h * 256 : (h + 1) * 256], x_ap[h * 256 : (h + 1) * 256, :]
            )
        # gpsimd SWDGE queue: tiny constants FIRST (bias gates the first qkT
        # adds), then weights in first-use order; wqk split so the q-half
        # lands before the k-half is needed.
        keep2 = const.tile([128, 2, 128], f16)   # keep[j, ix, i] = (j <= i)
        nc.gpsimd.dma_start(keep2[:], keep2_ap)
        bqk = const.tile([128, CB], f32)
        nc.gpsimd.dma_start(bqk[:], bqk_ap.rearrange("co p -> p co"))
        wqk_sb = p_w.tile([128, CB, CB * 128], f16)  # [c-part, cb, co*128+q]
        nc.gpsimd.dma_start(
            wqk_sb[:, :, 0 : 4 * 128],
            wqk_ap[:, 0 : 4 * 128].rearrange("(cb p) n -> p cb n", p=128),
        )
        nc.gpsimd.dma_start(
            wqk_sb[:, :, 4 * 128 : 8 * 128],
            wqk_ap[:, 4 * 128 : 8 * 128].rearrange("(cb p) n -> p cb n", p=128),
        )
        wv_sb = p_w.tile([128, CB, CL], f16)
        nc.gpsimd.dma_start(wv_sb[:], wv_ap.rearrange("(cb p) n -> p cb n", p=128))
        wp_sb = p_w.tile([128, DB, C], f16)
        nc.gpsimd.dma_start(wp_sb[:], wp_ap.rearrange("(db p) c -> p db c", p=128))

        # ---- PE warm-up: dep-free matmuls on a zeroed tile keep the PE busy
        # from t~5us so HAM un-throttles to 2.4 GHz before real work arrives,
        # and the array never sits cold waiting for the first x chunk.
        warm = const.tile([128, 512], f16)
        nc.vector.memset(warm[:], 0.0)
        wups = ps_u.tile([HD + 1, 512], f32, tag="u", name="wups")
        for _ in range(18):
            nc.tensor.matmul(
                wups[:], warm[:, 0 : HD + 1], warm[:], start=True, stop=True
            )

        # per-(co, tn) qkT tiles, per-jt v' tiles, per-tn yT tiles
        qkT = {}
        for co in range(CB):
            for tn in range(NIC):
                qkT[(co, tn)] = p_qkT.tile(
                    [128, 512], f16, tag=f"qkT_{co}_{tn}", name=f"qkT_{co}_{tn}"
                )
        vv = {}
        for jt in range(TT):
            vv[jt] = p_v.tile([128, HG, HD + 1], f16, tag=f"vv_{jt}", name=f"vv_{jt}")
            nc.vector.memset(vv[jt][:, :, HD : HD + 1], 1.0)
        yTn = {}
        for tn in range(NIC):
            yTn[tn] = p_yT.tile([128, DB, 512], f16, tag=f"yT_{tn}", name=f"yT_{tn}")

        def emit_qkT_group(co, tn):
            ps = ps_b.tile([128, 512], f32, tag="bps")
            for cb in range(CB):
                nc.tensor.matmul(
                    ps[:],
                    wqk_sb[:, cb, co * 128 : (co + 1) * 128],
                    xT[:, cb, tn * 512 : (tn + 1) * 512],
                    start=(cb == 0),
                    stop=(cb == CB - 1),
                )
            nc.vector.tensor_scalar_add(qkT[(co, tn)][:], ps[:], bqk[:, co : co + 1])

        def emit_vv_group(tt):
            ps = ps_b.tile([128, CL], f32, tag="bps")
            for cb in range(CB):
                nc.tensor.matmul(
                    ps[:],
                    xT[:, cb, tt * 128 : (tt + 1) * 128],
                    wv_sb[:, cb, :],
                    start=(cb == 0),
                    stop=(cb == CB - 1),
                )
            nc.vector.tensor_copy(
                vv[tt][:, :, 0:HD], ps.rearrange("p (h d) -> p h d", d=HD)
            )

        def emit_pair(u, ic):
            """Attention for head pair (2u, 2u+1) over i-chunk ic."""
            co_q = u
            co_k = 4 + u
            nj = 4 * ic + 4
            ups_e = ps_u.tile([HD + 1, 512], f32, tag="u", name=f"ue_{u}_{ic}")
            ups_o = ps_u.tile([HD + 1, 512], f32, tag="u", name=f"uo_{u}_{ic}")
            for jt in range(nj):
                m = jt % 4
                diag = ic == jt // 4
                lo = 128 * m if diag else 0
                psg = ps_sc.tile([128, 2, 512], f32, tag="sc")
                for ix in range(2):
                    r0 = 64 * ix
                    nc.tensor.matmul(
                        psg[:, ix, lo:512],
                        qkT[(co_k, jt // 4)][r0 : r0 + 64, m * 128 : (m + 1) * 128],
                        qkT[(co_q, ic)][r0 : r0 + 64, lo:512],
                        start=True,
                        stop=True,
                    )
                pt = p_p.tile([128, 2, 512], f16, tag="p")
                nc.scalar.activation(
                    pt[:, 0:2, lo:512], psg[:, 0:2, lo:512], Exp, scale=1.0 / np.sqrt(HD)
                )
                if diag:  # zero strictly-upper part of the diagonal block
                    nc.vector.tensor_mul(
                        pt[:, 0:2, lo : lo + 128],
                        pt[:, 0:2, lo : lo + 128],
                        keep2[:, 0:2, :],
                    )
                for ix, ups in ((0, ups_e), (1, ups_o)):
                    nc.tensor.matmul(
                        ups[:, lo:512],
                        vv[jt][:, 2 * u + ix, :],
                        pt[:, ix, lo:512],
                        start=(jt == 0),
                        stop=(jt == nj - 1),
                    )
            for ups, r0 in ((ups_e, 0), (ups_o, 64)):
                rs = p_rb.tile([1, 512], f32, tag="rs", name="rs")
                nc.vector.tensor_copy(rs[:], ups[HD : HD + 1, :])
                rr = p_rb.tile([1, 512], f32, tag="rr", name="rr")
                nc.vector.reciprocal_approx_fast(rr[:], rs[:])
                rb = p_rb.tile([HD, 512], f32, tag="rb", name="rb")
                nc.gpsimd.partition_broadcast(rb[:], rr[0:1, :], channels=HD)
                nc.vector.tensor_mul(
                    yTn[ic][r0 : r0 + HD, u, :], ups[0:HD, :], rb[:]
                )

        def emit_proj(tn, cos):
            for co in cos:
                psp = ps_b.tile([128, 512], f32, tag="bps")
                for db in range(DB):
                    nc.tensor.matmul(
                        psp[:],
                        wp_sb[:, db, co * 128 : (co + 1) * 128],
                        yTn[tn][:, db, :],
                        start=(db == 0),
                        stop=(db == DB - 1),
                    )
                ot = p_ost.tile([128, 512], f32, tag="ot")
                nc.vector.tensor_copy(ot[:], psp[:])
                nc.sync.dma_start(
                    outT_ap[co * 128 : (co + 1) * 128, tn * 512 : (tn + 1) * 512],
                    ot[:],
                )

        # ---- B(0) up front; then C(ic) with B(ic+1)/proj(ic-1) interleaved
        for co in range(CB):
            emit_qkT_group(co, 0)
        for tt in range(4):
            emit_vv_group(tt)

        for ic in range(NIC):
            for u in range(NP):
                emit_pair(u, ic)
                if ic < NIC - 1:
                    emit_qkT_group(2 * u, ic + 1)
                    emit_qkT_group(2 * u + 1, ic + 1)
                    emit_vv_group(4 * (ic + 1) + u)
                if ic >= 1 and u < 2:
                    emit_proj(ic - 1, [2 * u, 2 * u + 1])
            # second half of the previous chunk's projection lands after the
            # last pair: dep-free PE filler over the final normalize chain
            if ic >= 1:
                emit_proj(ic - 1, [4, 5, 6, 7])
        emit_proj(NIC - 1, range(CB))


def _build_program():
    nc = bacc.Bacc("TRN2", target_bir_lowering=False, debug=False, num_devices=N_CORES)
    aps = {
        "x": nc.dram_tensor("x", [T, C], f16, kind="ExternalInput").ap(),
        "wqk": nc.dram_tensor("wqk", [C, CB * 128], f16, kind="ExternalInput").ap(),
        "wv": nc.dram_tensor("wv", [C, CL], f16, kind="ExternalInput").ap(),
        "wp": nc.dram_tensor("wp", [CL, C], f16, kind="ExternalInput").ap(),
        "bqk": nc.dram_tensor("bqk", [CB, 128], f32, kind="ExternalInput").ap(),
        "keep2": nc.dram_tensor("keep2", [128, 2, 128], f16, kind="ExternalInput").ap(),
        "outT": nc.dram_tensor("outT", [C, T], f32, kind="ExternalOutput").ap(),
    }
    with tile.TileContext(nc) as tc:
        _emit(tc, aps)
    nc.compile()
    return nc


def get_program():
    if "nc" not in _PROG_CACHE:
        _PROG_CACHE["nc"] = _build_program()
    return _PROG_CACHE["nc"]


def _host_consts():
    j = np.arange(128)[:, None]
    i = np.arange(128)[None, :]
    keep = (j <= i).astype(np.float16)          # 1 => keep
    keep2 = np.stack([keep, keep], axis=1)      # [128, 2, 128]
    return np.ascontiguousarray(keep2)


def make_in_maps(x, W_attn, b_attn, W_proj):
    """Build the 8 per-core input maps. Core index = 2*batch + head_group."""
    keep2 = _host_consts()
    in_maps = []
    for core in range(N_CORES):
        b = core // 2
        g = core % 2
        wq = W_attn[:, g * CL : (g + 1) * CL]
        wk = W_attn[:, C + g * CL : C + (g + 1) * CL]
        wqk = np.concatenate([wq, wk], axis=1)  # [C, 1024], cols = co*128+q
        wv = W_attn[:, 2 * C + g * CL : 2 * C + (g + 1) * CL]
        bqk = np.concatenate(
            [b_attn[g * CL : (g + 1) * CL], b_attn[C + g * CL : C + (g + 1) * CL]]
        ).reshape(CB, 128)
        in_maps.append(
            {
                "x": np.ascontiguousarray(x[b]).astype(np.float16),
                "wqk": np.ascontiguousarray(wqk).astype(np.float16),
                "wv": np.ascontiguousarray(wv).astype(np.float16),
                "wp": np.ascontiguousarray(W_proj[g * CL : (g + 1) * CL, :]).astype(
                    np.float16
                ),
                "bqk": np.ascontiguousarray(bqk).astype(np.float32),
                "keep2": keep2,
            }
        )
    return in_maps


def run(x, W_attn, b_attn, W_proj, b_proj, trace=False):
    nc = get_program()
    in_maps = make_in_maps(x, W_attn, b_attn, W_proj)
    res = bass_utils.run_bass_kernel_spmd(
        nc, in_maps, core_ids=list(range(N_CORES)), trace=trace
    )
    # combine: out[b] = sum_g outT_{2b+g}^T + (bv_g @ Wp_g summed) + b_proj
    corr = b_proj.astype(np.float64).copy()
    for g in range(2):
        bv_g = b_attn[2 * C + g * CL : 2 * C + (g + 1) * CL]
        corr += bv_g.astype(np.float64) @ W_proj[g * CL : (g + 1) * CL, :].astype(
            np.float64
        )
    out = np.empty((B, T, C), np.float32)
    for b in range(B):
        acc = (
            res.results[2 * b]["outT"].T.astype(np.float64)
            + res.results[2 * b + 1]["outT"].T.astype(np.float64)
            + corr
        )
        out[b] = acc.astype(np.float32)
    return out, res


def kernel(x, W_attn, b_attn, W_proj, b_proj):
    x = np.asarray(x, np.float32)
    W_attn = np.asarray(W_attn, np.float32)
    b_attn = np.asarray(b_attn, np.float32)
    W_proj = np.asarray(W_proj, np.float32)
    b_proj = np.asarray(b_proj, np.float32)
    out, _ = run(x, W_attn, b_attn, W_proj, b_proj)
    return out


# revision 10
# speedup vs baseline: 1.2827x; 1.0428x over previous
"""Causal self-attention (B=4, T=2048, C=1024, H=16) on 8 Trainium2 NeuronCores.

Core index = 2*batch + head_group: each core owns one batch element and 8 of
the 16 heads (tensor-parallel split of c_attn output dim / c_proj input dim).
Each core emits a partial projection out^T [C, T]; the host sums the two
head-group partials per batch and adds the bias terms.

v2 (vs v1 baseline 358us):
  * Head-PAIR processing: even local head lives on SBUF partitions 0-63,
    odd head on 64-127. The two K=64 scores matmuls of a pair are emitted
    back-to-back with inferred tile_position (0,0)/(64,0) -> they run
    CONCURRENTLY in the PE array (row-group tiling), and their LDWEIGHTS
    cross-hide against the other row group's matmul.
  * No negI mask matmuls: causal masking is a post-exp DVE multiply of the
    128-wide diagonal block only (PE -21us, DVE +16us).
  * qkv projection (phase B), attention (phase C) and out-projection are
    interleaved in emission order so the Tile scheduler can fill PE stalls
    (C is ACT-exp-bound in bursts) with independent B/proj matmuls.
  * x DMA-transposes split across the two HWDGE queues (sync+scalar) and
    issued first: PE starts ~6us in instead of ~24us.
  * normalize reads U'/rowsum directly from PSUM (no staging copies).

fp16 datapath (fp32 PSUM accumulation everywhere, fp32 softmax denominator).

Per-core pipeline per head pair u (heads 2u, 2u+1), per 512-wide i-chunk ic:
  S^T[j, i] = k_h^T q_h  for both heads -> one psum tile [128, 2, 512]
  P = exp(S^T / 8)       (one ACT op per (jt, ic) sub, both heads)
  diagonal j-tile: P *= keep (DVE, precomputed lower-tri mask)
  U'^T [65, i] (+)= [v_h|1]^T P_h^T  over j-tiles (ones col => rowsum row 64)
  yT[u-block, i] = U'[0:64] * bcast(1/rowsum)
out^T = W_p^T yT -> fp32 psum -> DVE copy -> DMA
"""

import numpy as np

import concourse.bass as bass
import concourse.mybir as mybir
import concourse.tile as tile
from concourse import bacc, bass_utils

B, T, C, H = 4, 2048, 1024, 16
HD = C // H          # 64 head dim
N_CORES = 8
HG = H // 2          # 8 heads per core
CL = HG * HD         # 512 local width of q/k/v
TT = T // 128        # 16 t-tiles
CB = C // 128        # 8 c-tiles
DB = CL // 128       # 4 local-hd tiles
NIC = T // 512       # i-chunks (4)
NP = HG // 2         # head pairs per core (4)

f32 = mybir.dt.float32
f16 = mybir.dt.float16

_PROG_CACHE = {}


def _emit(tc, aps):
    nc = tc.nc
    Exp = mybir.ActivationFunctionType.Exp
    Copy = mybir.ActivationFunctionType.Copy

    x_ap = aps["x"]
    wqk_ap = aps["wqk"]
    wv_ap = aps["wv"]
    wp_ap = aps["wp"]
    bqk_ap = aps["bqk"]
    keep2_ap = aps["keep2"]
    outT_ap = aps["outT"]

    from contextlib import ExitStack

    with ExitStack() as outer:
        const = outer.enter_context(tc.tile_pool(name="const", bufs=1))
        p_xT = outer.enter_context(tc.tile_pool(name="xT", bufs=1))
        p_qkT = outer.enter_context(tc.tile_pool(name="qkT", bufs=1))
        p_v = outer.enter_context(tc.tile_pool(name="vv", bufs=1))
        p_yT = outer.enter_context(tc.tile_pool(name="yT", bufs=1))
        p_w = outer.enter_context(tc.tile_pool(name="wsb", bufs=1))
        p_p = outer.enter_context(tc.tile_pool(name="pp", bufs=8))
        p_rb = outer.enter_context(tc.tile_pool(name="rb", bufs=3))
        p_ost = outer.enter_context(tc.tile_pool(name="ost", bufs=4))
        ps_b = outer.enter_context(tc.tile_pool(name="ps_b", bufs=2, space="PSUM"))
        ps_sc = outer.enter_context(tc.tile_pool(name="ps_sc", bufs=2, space="PSUM"))
        ps_u = outer.enter_context(tc.tile_pool(name="ps_u", bufs=2, space="PSUM"))

        # ---- DMAs: x transposes are the PE critical path -> they own both
        # HWDGE queues (sync+scalar), with t-chunk 0 split across the two so
        # its descriptor GENERATION (~6us each) runs in parallel. All weight
        # loads go through the gpsimd SWDGE queue (cheap generation).
        # x transposes: XBAR transfer is fast (~14ns/16x128 tile); the cost is
        # per-instruction DGE latency, which pipelines across the two HWDGE
        # queues. Small first pieces unblock B(0) early.
        xT = p_xT.tile([128, CB, T], f16)
        nc.sync.dma_start_transpose(xT[:, :, 0:256], x_ap[0:256, :])
        nc.scalar.dma_start_transpose(xT[:, :, 256:512], x_ap[256:512, :])
        nc.sync.dma_start_transpose(xT[:, :, 512:1024], x_ap[512:1024, :])
        nc.scalar.dma_start_transpose(xT[:, :, 1024:1536], x_ap[1024:1536, :])
        nc.sync.dma_start_transpose(xT[:, :, 1536:2048], x_ap[1536:2048, :])
        # gpsimd SWDGE queue: tiny constants FIRST (bias gates the first qkT
        # adds), then weights in first-use order; wqk split so the q-half
        # lands before the k-half is needed.
        keep2 = const.tile([128, 2, 128], f16)   # keep[j, ix, i] = (j <= i)
        nc.gpsimd.dma_start(keep2[:], keep2_ap)
        bqk = const.tile([128, CB], f32)
        nc.gpsimd.dma_start(bqk[:], bqk_ap.rearrange("co p -> p co"))
        wqk_sb = p_w.tile([128, CB, CB * 128], f16)  # [c-part, cb, co*128+q]
        nc.gpsimd.dma_start(
            wqk_sb[:, :, 0 : 4 * 128],
            wqk_ap[:, 0 : 4 * 128].rearrange("(cb p) n -> p cb n", p=128),
        )
        nc.gpsimd.dma_start(
            wqk_sb[:, :, 4 * 128 : 8 * 128],
            wqk_ap[:, 4 * 128 : 8 * 128].rearrange("(cb p) n -> p cb n", p=128),
        )
        wv_sb = p_w.tile([128, CB, CL], f16)
        nc.gpsimd.dma_start(wv_sb[:], wv_ap.rearrange("(cb p) n -> p cb n", p=128))
        wp_sb = p_w.tile([128, DB, C], f16)
        nc.gpsimd.dma_start(wp_sb[:], wp_ap.rearrange("(db p) c -> p db c", p=128))

        # ---- PE warm-up: dep-free matmuls on a zeroed tile keep the PE busy
        # from t~5us so HAM un-throttles to 2.4 GHz before real work arrives,
        # and the array never sits cold waiting for the first x chunk.
        warm = const.tile([128, 512], f16)
        nc.vector.memset(warm[:], 0.0)
        wups = ps_u.tile([HD + 1, 512], f32, tag="u", name="wups")
        for _ in range(18):
            nc.tensor.matmul(
                wups[:], warm[:, 0 : HD + 1], warm[:], start=True, stop=True
            )

        # per-(co, tn) qkT tiles, per-jt v' tiles, per-tn yT tiles
        qkT = {}
        for co in range(CB):
            for tn in range(NIC):
                qkT[(co, tn)] = p_qkT.tile(
                    [128, 512], f16, tag=f"qkT_{co}_{tn}", name=f"qkT_{co}_{tn}"
                )
        vv = {}
        for jt in range(TT):
            vv[jt] = p_v.tile([128, HG, HD + 1], f16, tag=f"vv_{jt}", name=f"vv_{jt}")
            nc.vector.memset(vv[jt][:, :, HD : HD + 1], 1.0)
        yTn = {}
        for tn in range(NIC):
            yTn[tn] = p_yT.tile([128, DB, 512], f16, tag=f"yT_{tn}", name=f"yT_{tn}")

        def emit_qkT_group(co, tn):
            ps = ps_b.tile([128, 512], f32, tag="bps")
            for cb in range(CB):
                nc.tensor.matmul(
                    ps[:],
                    wqk_sb[:, cb, co * 128 : (co + 1) * 128],
                    xT[:, cb, tn * 512 : (tn + 1) * 512],
                    start=(cb == 0),
                    stop=(cb == CB - 1),
                )
            nc.vector.tensor_scalar_add(qkT[(co, tn)][:], ps[:], bqk[:, co : co + 1])

        def emit_vv_group(tt):
            ps = ps_b.tile([128, CL], f32, tag="bps")
            for cb in range(CB):
                nc.tensor.matmul(
                    ps[:],
                    xT[:, cb, tt * 128 : (tt + 1) * 128],
                    wv_sb[:, cb, :],
                    start=(cb == 0),
                    stop=(cb == CB - 1),
                )
            nc.vector.tensor_copy(
                vv[tt][:, :, 0:HD], ps.rearrange("p (h d) -> p h d", d=HD)
            )

        def normalize(ups, ic, u, r0):
            rs = p_rb.tile([1, 512], f32, tag="rs", name="rs")
            nc.vector.tensor_copy(rs[:], ups[HD : HD + 1, :])
            rr = p_rb.tile([1, 512], f32, tag="rr", name="rr")
            nc.vector.reciprocal_approx_fast(rr[:], rs[:])
            rb = p_rb.tile([HD, 512], f32, tag="rb", name="rb")
            nc.gpsimd.partition_broadcast(rb[:], rr[0:1, :], channels=HD)
            nc.vector.tensor_mul(yTn[ic][r0 : r0 + HD, u, :], ups[0:HD, :], rb[:])

        def emit_pair(u, ic):
            """Attention for head pair (2u, 2u+1) over i-chunk ic. The odd
            head's P@V lags LAG j-tiles so the even head's accumulator closes
            early and its normalize chain overlaps remaining PE work."""
            co_q = u
            co_k = 4 + u
            nj = 4 * ic + 4
            lag = min(3, nj - 1)
            ups_e = ps_u.tile([HD + 1, 512], f32, tag="u", name=f"ue_{u}_{ic}")
            ups_o = ps_u.tile([HD + 1, 512], f32, tag="u", name=f"uo_{u}_{ic}")
            pts = {}

            def av(ix, ups, jt):
                m = jt % 4
                lo = 128 * m if ic == jt // 4 else 0
                nc.tensor.matmul(
                    ups[:, lo:512],
                    vv[jt][:, 2 * u + ix, :],
                    pts[jt][:, ix, lo:512],
                    start=(jt == 0),
                    stop=(jt == nj - 1),
                )

            for jt in range(nj):
                m = jt % 4
                diag = ic == jt // 4
                lo = 128 * m if diag else 0
                psg = ps_sc.tile([128, 2, 512], f32, tag="sc")
                for ix in range(2):
                    r0 = 64 * ix
                    nc.tensor.matmul(
                        psg[:, ix, lo:512],
                        qkT[(co_k, jt // 4)][r0 : r0 + 64, m * 128 : (m + 1) * 128],
                        qkT[(co_q, ic)][r0 : r0 + 64, lo:512],
                        start=True,
                        stop=True,
                    )
                pt = p_p.tile([128, 2, 512], f16, tag="p")
                pts[jt] = pt
                nc.scalar.activation(
                    pt[:, 0:2, lo:512], psg[:, 0:2, lo:512], Exp, scale=1.0 / np.sqrt(HD)
                )
                if diag:  # zero strictly-upper part of the diagonal block
                    nc.vector.tensor_mul(
                        pt[:, 0:2, lo : lo + 128],
                        pt[:, 0:2, lo : lo + 128],
                        keep2[:, 0:2, :],
                    )
                av(0, ups_e, jt)
                if jt >= lag:
                    av(1, ups_o, jt - lag)
            normalize(ups_e, ic, u, 0)
            for jt in range(nj - lag, nj):
                av(1, ups_o, jt)
            normalize(ups_o, ic, u, 64)

        def emit_proj(tn, cos):
            for co in cos:
                psp = ps_b.tile([128, 512], f32, tag="bps")
                for db in range(DB):
                    nc.tensor.matmul(
                        psp[:],
                        wp_sb[:, db, co * 128 : (co + 1) * 128],
                        yTn[tn][:, db, :],
                        start=(db == 0),
                        stop=(db == DB - 1),
                    )
                ot = p_ost.tile([128, 512], f32, tag="ot")
                nc.vector.tensor_copy(ot[:], psp[:])
                nc.sync.dma_start(
                    outT_ap[co * 128 : (co + 1) * 128, tn * 512 : (tn + 1) * 512],
                    ot[:],
                )

        # ---- B(0) up front; then C(ic) with B(ic+1)/proj(ic-1) interleaved
        for co in range(CB):
            emit_qkT_group(co, 0)
        for tt in range(4):
            emit_vv_group(tt)

        for ic in range(NIC):
            for u in range(NP):
                emit_pair(u, ic)
                if ic < NIC - 1:
                    emit_qkT_group(2 * u, ic + 1)
                    emit_qkT_group(2 * u + 1, ic + 1)
                    emit_vv_group(4 * (ic + 1) + u)
                if ic >= 1:
                    emit_proj(ic - 1, [u])
            # second half of the previous chunk's projection lands after the
            # last pair: dep-free PE filler over the final normalize chain
            if ic >= 1:
                emit_proj(ic - 1, [4, 5, 6, 7])
        emit_proj(NIC - 1, range(CB))


def _build_program():
    nc = bacc.Bacc("TRN2", target_bir_lowering=False, debug=False, num_devices=N_CORES)
    aps = {
        "x": nc.dram_tensor("x", [T, C], f16, kind="ExternalInput").ap(),
        "wqk": nc.dram_tensor("wqk", [C, CB * 128], f16, kind="ExternalInput").ap(),
        "wv": nc.dram_tensor("wv", [C, CL], f16, kind="ExternalInput").ap(),
        "wp": nc.dram_tensor("wp", [CL, C], f16, kind="ExternalInput").ap(),
        "bqk": nc.dram_tensor("bqk", [CB, 128], f32, kind="ExternalInput").ap(),
        "keep2": nc.dram_tensor("keep2", [128, 2, 128], f16, kind="ExternalInput").ap(),
        "outT": nc.dram_tensor("outT", [C, T], f32, kind="ExternalOutput").ap(),
    }
    with tile.TileContext(nc) as tc:
        _emit(tc, aps)
    nc.compile()
    return nc


def get_program():
    if "nc" not in _PROG_CACHE:
        _PROG_CACHE["nc"] = _build_program()
    return _PROG_CACHE["nc"]


def _host_consts():
    j = np.arange(128)[:, None]
    i = np.arange(128)[None, :]
    keep = (j <= i).astype(np.float16)          # 1 => keep
    keep2 = np.stack([keep, keep], axis=1)      # [128, 2, 128]
    return np.ascontiguousarray(keep2)


def make_in_maps(x, W_attn, b_attn, W_proj):
    """Build the 8 per-core input maps. Core index = 2*batch + head_group."""
    keep2 = _host_consts()
    in_maps = []
    for core in range(N_CORES):
        b = core // 2
        g = core % 2
        wq = W_attn[:, g * CL : (g + 1) * CL]
        wk = W_attn[:, C + g * CL : C + (g + 1) * CL]
        wqk = np.concatenate([wq, wk], axis=1)  # [C, 1024], cols = co*128+q
        wv = W_attn[:, 2 * C + g * CL : 2 * C + (g + 1) * CL]
        bqk = np.concatenate(
            [b_attn[g * CL : (g + 1) * CL], b_attn[C + g * CL : C + (g + 1) * CL]]
        ).reshape(CB, 128)
        in_maps.append(
            {
                "x": np.ascontiguousarray(x[b]).astype(np.float16),
                "wqk": np.ascontiguousarray(wqk).astype(np.float16),
                "wv": np.ascontiguousarray(wv).astype(np.float16),
                "wp": np.ascontiguousarray(W_proj[g * CL : (g + 1) * CL, :]).astype(
                    np.float16
                ),
                "bqk": np.ascontiguousarray(bqk).astype(np.float32),
                "keep2": keep2,
            }
        )
    return in_maps


def run(x, W_attn, b_attn, W_proj, b_proj, trace=False):
    nc = get_program()
    in_maps = make_in_maps(x, W_attn, b_attn, W_proj)
    res = bass_utils.run_bass_kernel_spmd(
        nc, in_maps, core_ids=list(range(N_CORES)), trace=trace
    )
    # combine: out[b] = sum_g outT_{2b+g}^T + (bv_g @ Wp_g summed) + b_proj
    corr = b_proj.astype(np.float64).copy()
    for g in range(2):
        bv_g = b_attn[2 * C + g * CL : 2 * C + (g + 1) * CL]
        corr += bv_g.astype(np.float64) @ W_proj[g * CL : (g + 1) * CL, :].astype(
            np.float64
        )
    out = np.empty((B, T, C), np.float32)
    for b in range(B):
        acc = (
            res.results[2 * b]["outT"].T.astype(np.float64)
            + res.results[2 * b + 1]["outT"].T.astype(np.float64)
            + corr
        )
        out[b] = acc.astype(np.float32)
    return out, res


def kernel(x, W_attn, b_attn, W_proj, b_proj):
    x = np.asarray(x, np.float32)
    W_attn = np.asarray(W_attn, np.float32)
    b_attn = np.asarray(b_attn, np.float32)
    W_proj = np.asarray(W_proj, np.float32)
    b_proj = np.asarray(b_proj, np.float32)
    out, _ = run(x, W_attn, b_attn, W_proj, b_proj)
    return out
